# revision 33
# baseline (speedup 1.0000x reference)
"""Trainium2 Bass kernel for nn_MultiHeadAttention_61701500175237.

Sharding: 8 cores = 2 batches x 4 head-groups (4 heads each).
Each core computes Q/K/V projections for its (batch, 4-head) slice, RoPE,
causal attention, and a partial o_proj covering the full d_model; the host
sums the 4 partials per batch (the "all-reduce" of the hint, done at gather
time since the partials are independent and the harness gathers on host).

Device dataflow (per core, transposed-attention layout, bf16 matmul
operands with fp32 PSUM accumulation), FUSED single-phase schedule:
the per-core kernel is ACT(exp)-bound in attention (~81us of exp) and
PE-bound in projections (~41us); a single fused emission lets the Tile
scheduler run QKV projection of token-block t+1 and o_proj of q-tile t-1
on the PE underneath the exp stream of q-tile t, so no engine idles and
the PE HAM clock stays warm without dummy-matmul fillers.

  - xT [1024, 2048] bf16 resident in SBUF (d on partitions: no on-device
    transposes anywhere); 8 whole-row-chunk DMAs at startup
  - QT/KT [j, tok] = W-shard.T (stationary) @ xT (moving)
  - RoPE in [j, tok] layout: weight rows host-permuted per head to
    [evens 0:16 | odds 0:16 | evens 16:32 | odds 16:32] so the rotation
    partner lives 16 partitions away -> one DVE stream_shuffle; the
    PSUM->SBUF cast copy also on DVE (ACT is the global bottleneck)
  - logits^T [k, q] = KT-slice (stationary, K=64) @ QT-slice; two heads run
    concurrently in PE row-groups 0:64 / 64:128; causally dead columns
    trimmed, diagonal 128x128 block masked by a 0/1 multiply post-exp
  - P = exp(0.125 * logits^T) on ACT straight out of PSUM (the only ACT
    work in the kernel; one activation table load)
  - attn^T [d, q] (+ sumexp row) = [V | ones] (stationary) @ P
  - 1/Z via DVE reciprocal_approx_fast (replaces the ACT ln/exp chain),
    partition-broadcast on GpSimd, normalize on DVE
  - o_proj: out[tok, n] = attn^T chunk (stationary) @ Wo-shard.T (moving),
    fp32 partial evicted by DVE and DMAd to DRAM
"""

import sys

if "/opt/trn_rl_repo" not in sys.path:
    sys.path.insert(0, "/opt/trn_rl_repo")

import numpy as np
import ml_dtypes

import concourse.bass as bass  # noqa: F401
import concourse.tile as tile
from concourse import bacc, mybir

F32 = mybir.dt.float32
BF16 = mybir.dt.bfloat16
AF = mybir.ActivationFunctionType
NPBF16 = np.dtype(ml_dtypes.bfloat16)

B = 2
S = 2048
D_MODEL = 1024
N_HEADS = 16
D_K = 64
THETA = 10000.0

H_PER = 4          # heads per core
JW = H_PER * D_K   # 256: per-core projection width
N_CORES = 8
VSTRIDE = D_K + 1  # V tile col stride per head (64 data + 1 ones)
NDC = D_MODEL // 128  # 8 d-chunks

SWAP_MASK = list(range(16, 32)) + list(range(16))  # exchange 16-halves


def _act(nc, out, in_, func, scale=1.0):
    """ACT activation: out = func(in_*scale)."""
    return nc.scalar.activation(out, in_, func, bias=0.0, scale=float(scale))


_tables_pinned = False


def _pin_act_table():
    """Make every ACT func we emit resolve to the single table
    natural_log_exp_and_others so exactly one table load happens."""
    global _tables_pinned
    if _tables_pinned:
        return
    _tables_pinned = True
    import concourse.bacc as bacc_mod

    orig = bacc_mod.get_activation_tables
    keep = "natural_log_exp_and_others"
    ours = {AF.Exp, AF.Ln, AF.Copy, AF.Identity}

    def pinned(arch):
        t = orig(arch)
        return {
            name: (funcs if name == keep else funcs - ours)
            for name, funcs in t.items()
        }

    bacc_mod.get_activation_tables = pinned


def _build_program():
    _pin_act_table()
    nc = bacc.Bacc("TRN2", target_bir_lowering=False, debug=False)

    xT = nc.dram_tensor("xT", [D_MODEL, S], BF16, kind="ExternalInput")
    wq = nc.dram_tensor("wq", [D_MODEL, JW], BF16, kind="ExternalInput")
    wk = nc.dram_tensor("wk", [D_MODEL, JW], BF16, kind="ExternalInput")
    wv = nc.dram_tensor("wv", [D_MODEL, JW], BF16, kind="ExternalInput")
    wo = nc.dram_tensor("wo", [JW, D_MODEL], BF16, kind="ExternalInput")
    cost = nc.dram_tensor("cost", [128, S], BF16, kind="ExternalInput")
    sint = nc.dram_tensor("sint", [128, S], BF16, kind="ExternalInput")
    maskt = nc.dram_tensor("maskt", [128, 128], BF16, kind="ExternalInput")
    outp = nc.dram_tensor("outp", [S, D_MODEL], BF16, kind="ExternalOutput")

    with tile.TileContext(nc) as tc:
        _body(tc, xT, wq, wk, wv, wo, cost, sint, maskt, outp)
    nc.compile()
    return nc


def _body(tc, xT, wq, wk, wv, wo, cost, sint, maskt, outp):
    nc = tc.nc

    with (
        tc.tile_pool(name="const", bufs=1) as cpool,
        tc.tile_pool(name="big", bufs=1) as bpool,
        tc.tile_pool(name="ropep", bufs=3) as ropep,
        tc.tile_pool(name="pp", bufs=4) as pp,
        tc.tile_pool(name="np_", bufs=2) as npool,
        tc.tile_pool(name="op", bufs=3) as op,
        tc.tile_pool(name="ps", space="PSUM", bufs=1) as psp,
    ):
        # --- resident inputs: whole-tensor DMAs, spread across queues so
        # the first projection chain's operands (x chunk 0, wq) land first.
        xfull = bpool.tile([128, NDC, S], BF16, name="xfull")
        wq_sb = cpool.tile([128, NDC, JW], BF16, name="wq_sb")
        wk_sb = cpool.tile([128, NDC, JW], BF16, name="wk_sb")
        wv_sb = cpool.tile([128, NDC, JW], BF16, name="wv_sb")
        wo_sb = cpool.tile([128, 2, D_MODEL], BF16, name="wo_sb")
        cs_sb = cpool.tile([128, 2, S], BF16, name="cs_sb")
        mask_sb = cpool.tile([128, 128], BF16, name="mask_sb")

        # priority: x chunks + wq/wk + cos/sin(token-block 0) first (gates the
        # first Q/K chains + rope), then wv, remaining cos/sin, wo, mask.
        # NOTHING on the scalar queue: a dma_start issue occupies that
        # engine's sequencer, and ACT (exp) is the critical engine.
        # x is DMAd per (d-chunk, token-block) so the first Q/K chains (which
        # only contract block-0 columns) start ~4us in instead of waiting for
        # the whole 4MB of x.
        nc.gpsimd.dma_start(wq_sb[:], wq.rearrange("(c p) j -> p c j", p=128))
        nc.sync.dma_start(wk_sb[:], wk.rearrange("(c p) j -> p c j", p=128))
        qs = [nc.sync, nc.gpsimd]
        for dc in range(NDC):
            bsl = slice(0, 512)
            qs[dc % 2].dma_start(xfull[:, dc, bsl],
                                 xT[dc * 128:(dc + 1) * 128, bsl])
        nc.sync.dma_start(cs_sb[:, 0, 0:512], cost[:, 0:512])
        nc.gpsimd.dma_start(cs_sb[:, 1, 0:512], sint[:, 0:512])
        for dc in range(NDC):
            bsl = slice(512, 1024)
            qs[(dc + 1) % 2].dma_start(xfull[:, dc, bsl],
                                       xT[dc * 128:(dc + 1) * 128, bsl])
        nc.gpsimd.dma_start(wv_sb[:], wv.rearrange("(c p) j -> p c j", p=128))
        nc.sync.dma_start(mask_sb[:], maskt[:])
        for b in range(2, 4):
            for dc in range(NDC):
                bsl = slice(b * 512, (b + 1) * 512)
                qs[(dc + b) % 2].dma_start(xfull[:, dc, bsl],
                                           xT[dc * 128:(dc + 1) * 128, bsl])
        for tt in range(1, 4):
            tsl = slice(tt * 512, (tt + 1) * 512)
            nc.sync.dma_start(cs_sb[:, 0, tsl], cost[:, tsl])
            nc.gpsimd.dma_start(cs_sb[:, 1, tsl], sint[:, tsl])
        nc.gpsimd.dma_start(wo_sb[:], wo.rearrange("(c p) n -> p c n", p=128))

        # --- persistent activations ---
        qt_sb = bpool.tile([128, 2, S], BF16, name="qt_sb")   # [j, jg, tok]
        kt_sb = bpool.tile([128, 2, S], BF16, name="kt_sb")
        v_sb = bpool.tile([128, S // 128, H_PER * VSTRIDE], BF16, name="v_sb")
        at_sb = bpool.tile([128, 2, S], BF16, name="at_sb")   # attn^T normalized

        # ones columns for the fused softmax denominator
        for h in range(H_PER):
            nc.vector.memset(v_sb[:, :, h * VSTRIDE + D_K], 1.0)

        # HAM warmup scratch: dependency-free matmuls through a pw slot keep
        # the PE busy while the startup DMAs land.
        wsc = cpool.tile([128, 512], BF16, name="wsc")
        nc.vector.memset(wsc[:], 0.0)

        def pe_filler(n=1, free=512):
            wps = psp.tile([128, 512], F32, name=f"wps{pe_filler.i}",
                           tag="pw", bufs=2)
            pe_filler.i += 1
            for _ in range(n):
                nc.tensor.matmul(wps[:, 0:free], wsc[:, 0:128],
                                 wsc[:, 0:free], start=True, stop=True)
        pe_filler.i = 0

        # ---------------- pump machinery ----------------
        # proj/oproj work is emitted as small closures ("units") drained a
        # couple at a time inside the attention kt loop, so the PE queue
        # never holds a long projection run in front of the logits that
        # feed the ACT exp stream (the critical engine).
        pump_q = []   # entries: (label, closure)

        def pump(n, backstop=False):
            for _ in range(n):
                if not pump_q:
                    if backstop:
                        pe_filler(1)   # keep HAM at 8/8 in ACT-bound tail
                    return
                pump_q.pop(0)[1]()

        def queue(lbl, units):
            pump_q.extend((lbl, u) for u in units)

        def flush_label(lbl):
            while any(l == lbl for l, _ in pump_q):
                pump_q.pop(0)[1]()

        def flush_pump():
            while pump_q:
                pump_q.pop(0)[1]()

        def qk_pair(tt, jg):
            """Unit list: Q+K projection chains for (token block tt, head
            pair jg), then their RoPE."""
            tsl = slice(tt * 512, (tt + 1) * 512)
            st = {}

            def mk_mm(dcs):
                def u():
                    if "chains" not in st:
                        st["chains"] = []
                        for wsb, dst, pnm in ((wq_sb, qt_sb, "q"),
                                              (wk_sb, kt_sb, "k")):
                            ps = psp.tile([128, 512], F32,
                                          name=f"ps{pnm}{tt}{jg}",
                                          tag="pw", bufs=2)
                            st["chains"].append((ps, wsb, dst))
                    for dc in dcs:
                        for ps, wsb, dst in st["chains"]:
                            nc.tensor.matmul(
                                ps[:],
                                wsb[:, dc, jg * 128:(jg + 1) * 128],
                                xfull[:, dc, tsl],
                                start=(dc == 0), stop=(dc == NDC - 1),
                                skip_group_check=True,
                            )
                return u

            def rope_u():
                for ci, (ps, wsb, dst) in enumerate(st["chains"]):
                    # RoPE: dst = eq*cos + shuffle16(eq)*sin', value and its
                    # shuffle packed so one strided mul covers both products
                    eq = ropep.tile([128, 2, 512], BF16,
                                    name=f"eq{tt}{jg}{ci}", tag="eq")
                    nc.vector.tensor_copy(eq[:, 0, :], ps[:])
                    nc.vector.stream_shuffle(eq[:, 1, :], eq[:, 0, :],
                                             SWAP_MASK)
                    t12 = ropep.tile([128, 2, 512], BF16,
                                     name=f"t12{tt}{jg}{ci}", tag="t12")
                    nc.vector.tensor_mul(t12[:], eq[:], cs_sb[:, :, tsl])
                    nc.vector.tensor_add(dst[:, jg, tsl],
                                         t12[:, 0, :], t12[:, 1, :])

            # 2-dc units: fine enough to interleave with attention, coarse
            # enough that the PE reorder window still hides LDWEIGHTS
            return [mk_mm([0, 1]), mk_mm([2, 3]), mk_mm([4, 5]),
                    mk_mm([6, 7]), rope_u]

        def v_pair(tt, stp):
            """Unit list: V projection for two 128-token subtiles."""
            st = {}

            def mk_mm(dcs):
                def u():
                    if "vts" not in st:
                        st["vts"] = []
                        for sti in range(2):
                            s2 = 2 * stp + sti
                            psv = psp.tile([128, JW], F32,
                                           name=f"psv{tt}{s2}",
                                           tag="pw", bufs=2)
                            st["vts"].append((s2, psv))
                    for dc in dcs:
                        for s2, psv in st["vts"]:
                            nc.tensor.matmul(
                                psv[:],
                                xfull[:, dc, tt * 512 + s2 * 128:
                                      tt * 512 + (s2 + 1) * 128],
                                wv_sb[:, dc, :],
                                start=(dc == 0), stop=(dc == NDC - 1),
                                skip_group_check=True,
                            )
                return u

            def ev_u():
                for s2, psv in st["vts"]:
                    ktile = tt * 4 + s2
                    # one strided copy fills all 4 heads' 64-col runs
                    nc.vector.tensor_copy(
                        v_sb[:, ktile, :]
                        .rearrange("p (h s) -> p h s", h=H_PER)[:, :, 0:D_K],
                        psv[:].rearrange("p (h s) -> p h s", h=H_PER),
                    )

            return [mk_mm([0, 1]), mk_mm([2, 3]), mk_mm([4, 5]),
                    mk_mm([6, 7]), ev_u]

        def attention(hp, qt, pump_n=2, backstop=False, tail_fill=False):
            qsl = slice(qt * 512, (qt + 1) * 512)
            nkt = 4 * qt + 4
            nfull = 4 * qt
            pat = psp.tile([D_K + 1, 2, 512], F32,
                           name=f"pat{hp}{qt}", tag="pat", bufs=1)
            for kt in range(nkt):
                r = kt - nfull
                c0 = 128 * r if r >= 0 else 0
                psl = psp.tile([128, 2, 512], F32,
                               name=f"psl{hp}{qt}{kt}", tag="psl", bufs=2)
                p = pp.tile([128, 2, 512], BF16,
                            name=f"p{hp}{qt}{kt}", tag="p", bufs=6)
                for hh in range(2):
                    rows = slice(hh * 64, hh * 64 + 64)
                    nc.tensor.matmul(
                        psl[:, hh, c0:],
                        kt_sb[rows, hp, kt * 128:(kt + 1) * 128],
                        qt_sb[rows, hp, qt * 512 + c0:(qt + 1) * 512],
                        start=True, stop=True,
                    )
                _act(nc, p[:, :, c0:], psl[:, :, c0:], AF.Exp, scale=0.125)
                if r >= 0:
                    nc.vector.tensor_mul(
                        p[:, :, c0:c0 + 128],
                        p[:, :, c0:c0 + 128],
                        mask_sb[:, None, :].broadcast_to((128, 2, 128)),
                    )
                for hh in range(2):
                    h = 2 * hp + hh
                    nc.tensor.matmul(
                        pat[:, hh, c0:],
                        v_sb[:, kt, h * VSTRIDE:h * VSTRIDE + VSTRIDE],
                        p[:, hh, c0:],
                        start=(kt == 0), stop=(kt == nkt - 1),
                        skip_group_check=True,
                    )
                pump(pump_n, backstop)
            # normalize: at = pat[0:64] * bcast(exp(-ln pat[64])); ln and exp
            # share one activation table so no table reloads
            rf = npool.tile([D_K + 1, 2, 512], F32, name=f"rf{hp}{qt}", tag="rf")
            _act(nc, rf[64:65, :, :], pat[64:65, :, :], AF.Ln)
            if tail_fill:
                pe_filler(3)   # keep HAM at 8/8 through the last normalize
            rz = npool.tile([D_K + 1, 2, 512], F32, name=f"rz{hp}{qt}", tag="rz")
            _act(nc, rz[64:65, :, :], rf[64:65, :, :], AF.Exp, scale=-1.0)
            r0 = npool.tile([1, 2, 512], F32, name=f"r0{hp}{qt}", tag="r0")
            nc.sync.dma_start(r0[:], rz[64:65, :, :])
            if tail_fill:
                pe_filler(3)
            rb = npool.tile([64, 2, 512], F32, name=f"rb{hp}{qt}", tag="rb")
            nc.gpsimd.partition_broadcast(rb[:], r0[:])
            if tail_fill:
                pe_filler(3)
            nc.vector.tensor_mul(
                at_sb[0:64, hp, qsl], pat[0:64, 0, :], rb[:, 0, :]
            )
            tmp = npool.tile([64, 512], BF16, name=f"att{hp}{qt}", tag="att")
            nc.vector.tensor_mul(tmp[:], pat[0:64, 1, :], rb[:, 1, :])
            nc.sync.dma_start(at_sb[64:128, hp, qsl], tmp[:])
            pump(pump_n, backstop)

        def oproj_units(qt):
            units = []
            for tb in range(4 * qt, 4 * qt + 4):
                st = {}
                for nd in range(2):
                    def u(tb=tb, nd=nd, st=st):
                        rsl = slice(tb * 128, (tb + 1) * 128)
                        if nd == 0:
                            st["oev"] = op.tile([128, D_MODEL], BF16,
                                                name=f"oev{tb}", tag="oev")
                        oev = st["oev"]
                        pso = psp.tile([128, 512], F32, name=f"pso{tb}{nd}",
                                       tag="pw", bufs=2)
                        for hc in range(2):
                            nc.tensor.matmul(
                                pso[:],
                                at_sb[:, hc, rsl],
                                wo_sb[:, hc, nd * 512:(nd + 1) * 512],
                                start=(hc == 0), stop=(hc == 1),
                                skip_group_check=True,
                            )
                        nc.vector.tensor_copy(oev[:, nd * 512:(nd + 1) * 512],
                                              pso[:])
                        if nd == 1:
                            (nc.sync if tb % 2 == 0 else nc.gpsimd).dma_start(
                                outp[rsl, :], oev[:])
                    units.append(u)
            return units

        def oproj_tail(qt, hc):
            # two-pass tail: the hc=0 half depends only on the first
            # head-pair's normalize, so it runs under the second pair's
            # attention (as pump units); hc=1 lands after, merged on DVE.
            units = []
            for tb in range(4 * qt, 4 * qt + 4):
                def u(tb=tb):
                    rsl = slice(tb * 128, (tb + 1) * 128)
                    if hc == 0:
                        oev = op.tile([128, D_MODEL], F32, name=f"oevt{tb}",
                                      tag="oevt", bufs=4)
                        oproj_tail.oevs[tb] = oev
                    else:
                        oev = oproj_tail.oevs[tb]
                        oevb = op.tile([128, D_MODEL], BF16,
                                       name=f"oevb{tb}", tag="oevb", bufs=4)
                    for nd in range(2):
                        pso = psp.tile([128, 512], F32,
                                       name=f"psot{tb}{nd}{hc}",
                                       tag="pw", bufs=2)
                        nc.tensor.matmul(
                            pso[:],
                            at_sb[:, hc, rsl],
                            wo_sb[:, hc, nd * 512:(nd + 1) * 512],
                            start=True, stop=True, skip_group_check=True,
                        )
                        if hc == 0:
                            nc.vector.tensor_copy(
                                oev[:, nd * 512:(nd + 1) * 512], pso[:])
                        else:
                            nc.vector.tensor_add(
                                oevb[:, nd * 512:(nd + 1) * 512],
                                oev[:, nd * 512:(nd + 1) * 512], pso[:])
                    if hc == 1:
                        (nc.sync if tb % 2 == 0 else nc.gpsimd).dma_start(
                            outp[rsl, :], oevb[:])
                units.append(u)
            return units
        oproj_tail.oevs = {}

        # ---------------- fused schedule ----------------
        # proj(0)'s first half runs eagerly (gates the very first exp);
        # everything else drains through the pump inside attention loops at
        # the latest dependency-safe point: qk(qt,0) before attention(0,qt),
        # qk(qt,1) before attention(1,qt), V(qt) by the diagonal PV steps,
        # and all of o_proj inside qt=3 (the ACT-bound tail's PE slack).
        nq = S // 512
        # startup HAM warm-up: short N=256 fillers — enough sustained PE
        # activity to release the clock gate (~3.4us) without queueing so
        # much always-ready work that the first projection chains (gated on
        # the x DMA) get pushed behind it at cold clock.
        for _ in range(12):
            pe_filler(2, free=256)
        for u in qk_pair(0, 0):
            u()
        for u in v_pair(0, 0) + v_pair(0, 1):
            u()
        queue("qk01", qk_pair(0, 1))
        attention(0, 0, pump_n=2)
        flush_label("qk01")
        queue("qk10", qk_pair(1, 0))
        attention(1, 0, pump_n=2)
        flush_label("qk10")
        queue("v1", v_pair(1, 0) + v_pair(1, 1))
        queue("qk11", qk_pair(1, 1))
        attention(0, 1, pump_n=2)
        flush_label("qk11")
        queue("qk20", qk_pair(2, 0))
        attention(1, 1, pump_n=2)
        flush_label("qk20")
        queue("v2", v_pair(2, 0) + v_pair(2, 1))
        queue("qk21", qk_pair(2, 1))
        attention(0, 2, pump_n=2)
        flush_label("qk21")
        queue("qk30", qk_pair(3, 0))
        attention(1, 2, pump_n=2)
        flush_label("qk30")
        queue("v3", v_pair(3, 0) + v_pair(3, 1))
        queue("qk31", qk_pair(3, 1))
        attention(0, nq - 1, pump_n=1, backstop=True)
        flush_label("qk31")
        queue("op0", oproj_units(0))
        queue("op1", oproj_units(1))
        queue("op2", oproj_units(2))
        queue("tail0", oproj_tail(nq - 1, 0))
        attention(1, nq - 1, pump_n=1, backstop=True, tail_fill=True)
        flush_pump()
        pe_filler(6)   # bridge the final normalize chain
        for u in oproj_tail(nq - 1, 1):
            u()


# ---------------------------------------------------------------------------
# host-side sharding / tables
# ---------------------------------------------------------------------------

def _head_perm_and_freq():
    """Within-head row order [e0..e15 | o0..o15 | e16..e31 | o16..o31]
    (e_i = dim 2i, o_i = dim 2i+1) so the rope partner is 16 partitions away
    inside one 32-partition quadrant. Returns (perm, freq_idx, sin_sign)."""
    e = np.arange(0, D_K, 2)   # evens: x1, freq i = 0..31
    o = np.arange(1, D_K, 2)   # odds:  x2
    perm = np.concatenate([e[:16], o[:16], e[16:], o[16:]])
    freq = np.concatenate([np.arange(16), np.arange(16),
                           np.arange(16, 32), np.arange(16, 32)])
    sign = np.concatenate([-np.ones(16), np.ones(16),
                           -np.ones(16), np.ones(16)])
    return perm, freq, sign


def _rope_tables():
    half = D_K // 2
    inv_freq = THETA ** (-np.arange(half, dtype=np.float64) * 2.0 / D_K)
    ang = np.arange(S, dtype=np.float64)[None, :] * inv_freq[:, None]  # [32, S]
    cos32 = np.cos(ang)
    sin32 = np.sin(ang)
    _, freq, sign = _head_perm_and_freq()
    cos64 = cos32[freq]                      # [64, S]
    sin64 = sin32[freq] * sign[:, None]      # [64, S]
    cos128 = np.tile(cos64, (2, 1)).astype(NPBF16)
    sin128 = np.tile(sin64, (2, 1)).astype(NPBF16)
    return cos128, sin128


def _mask_table():
    kl = np.arange(128)[:, None]
    ql = np.arange(128)[None, :]
    return np.ascontiguousarray((ql >= kl).astype(NPBF16))


_nc_cache = None


def _get_nc():
    global _nc_cache
    if _nc_cache is None:
        _nc_cache = _build_program()
    return _nc_cache


def make_in_maps(x, Wq, Wk, Wv, Wo):
    x = np.asarray(x, dtype=np.float32)
    Wq = np.asarray(Wq, dtype=np.float32)
    Wk = np.asarray(Wk, dtype=np.float32)
    Wv = np.asarray(Wv, dtype=np.float32)
    Wo = np.asarray(Wo, dtype=np.float32)

    cos128, sin128 = _rope_tables()
    mask = _mask_table()
    perm, _, _ = _head_perm_and_freq()

    in_maps = []
    for c in range(N_CORES):
        b = c // 4
        hg = c % 4
        heads = np.arange(hg * H_PER, (hg + 1) * H_PER)
        rows_plain = (heads[:, None] * D_K + np.arange(D_K)[None, :]).reshape(-1)
        rows_perm = (heads[:, None] * D_K + perm[None, :]).reshape(-1)
        in_maps.append({
            "xT": np.ascontiguousarray(x[b].T).astype(NPBF16),
            "wq": np.ascontiguousarray(Wq[rows_perm, :].T).astype(NPBF16),
            "wk": np.ascontiguousarray(Wk[rows_perm, :].T).astype(NPBF16),
            "wv": np.ascontiguousarray(Wv[rows_plain, :].T).astype(NPBF16),
            "wo": np.ascontiguousarray(Wo[:, rows_plain].T).astype(NPBF16),
            "cost": cos128,
            "sint": sin128,
            "maskt": mask,
        })
    return in_maps


def gather_output(results):
    outs = [np.asarray(r["outp"], dtype=np.float32) for r in results]
    out = np.stack([
        outs[0] + outs[1] + outs[2] + outs[3],
        outs[4] + outs[5] + outs[6] + outs[7],
    ])
    return out.reshape(B, S, D_MODEL)


def _install_ntff_hook():
    """Provide antenv.axon_hooks + register the ctypes NTFF profile hook.

    The agent image's antenv package lacks axon_hooks, so trace=True under
    axon crashes on import. Recreate the tiny get/set module and drive
    profiling via direct ctypes calls into libaxon_pjrt.so (same ABI as
    trn_boot._ntff_profile_via_ctypes)."""
    import types
    import ctypes
    import contextlib

    if "antenv.axon_hooks" not in sys.modules:
        mod = types.ModuleType("antenv.axon_hooks")
        mod._hook = None

        def set_axon_ntff_profile_hook(h):
            mod._hook = h

        def get_axon_ntff_profile_hook():
            return mod._hook

        mod.set_axon_ntff_profile_hook = set_axon_ntff_profile_hook
        mod.get_axon_ntff_profile_hook = get_axon_ntff_profile_hook
        sys.modules["antenv.axon_hooks"] = mod
        import antenv

        antenv.axon_hooks = mod

    hooks = sys.modules["antenv.axon_hooks"]
    if hooks.get_axon_ntff_profile_hook() is not None:
        return

    so_path = "/opt/axon/libaxon_pjrt.so"
    try:
        lib = ctypes.CDLL(so_path)
    except OSError:
        return
    if not hasattr(lib, "axon_start_nrt_profile"):
        return
    lib.axon_start_nrt_profile.argtypes = [
        ctypes.POINTER(ctypes.c_int64), ctypes.c_size_t,
    ]
    lib.axon_start_nrt_profile.restype = ctypes.c_int64
    lib.axon_stop_nrt_profile.argtypes = [ctypes.c_char_p]
    lib.axon_stop_nrt_profile.restype = ctypes.c_int64

    @contextlib.contextmanager
    def _hook(output_dir, device_ids):
        import jax

        jax.devices()
        if device_ids:
            ids = (ctypes.c_int64 * len(device_ids))(*device_ids)
            rc = lib.axon_start_nrt_profile(ids, len(device_ids))
        else:
            rc = lib.axon_start_nrt_profile(None, 0)
        if rc != 0:
            raise RuntimeError(f"axon_start_nrt_profile rc={rc}")
        try:
            yield
        finally:
            n = lib.axon_stop_nrt_profile(str(output_dir).encode())
            print(f"profile: {n} file(s) written to {output_dir}")

    hooks.set_axon_ntff_profile_hook(_hook)


def kernel(x, Wq, Wk, Wv, Wo, _trace=False, _trace_cores=None):
    from concourse.bass_utils import run_bass_kernel_spmd

    if _trace:
        _install_ntff_hook()
    nc = _get_nc()
    in_maps = make_in_maps(x, Wq, Wk, Wv, Wo)
    res = run_bass_kernel_spmd(
        nc, in_maps, list(range(N_CORES)),
        trace=_trace, trace_cores=_trace_cores,
    )
    out = gather_output(res.results)
    if _trace:
        kernel.last_results = res
    return out


# revision 37
# speedup vs baseline: 1.0332x; 1.0332x over previous
"""Trainium2 Bass kernel for nn_MultiHeadAttention_61701500175237.

Sharding: 8 cores = 2 batches x 4 head-groups (4 heads each).
Each core computes Q/K/V projections for its (batch, 4-head) slice, RoPE,
causal attention, and a partial o_proj covering the full d_model; the host
sums the 4 partials per batch (the "all-reduce" of the hint, done at gather
time since the partials are independent and the harness gathers on host).

Device dataflow (per core, transposed-attention layout, bf16 matmul
operands with fp32 PSUM accumulation), FUSED single-phase schedule:
the per-core kernel is ACT(exp)-bound in attention (~81us of exp) and
PE-bound in projections (~41us); a single fused emission lets the Tile
scheduler run QKV projection of token-block t+1 and o_proj of q-tile t-1
on the PE underneath the exp stream of q-tile t, so no engine idles and
the PE HAM clock stays warm without dummy-matmul fillers.

  - xT [1024, 2048] bf16 resident in SBUF (d on partitions: no on-device
    transposes anywhere); 8 whole-row-chunk DMAs at startup
  - QT/KT [j, tok] = W-shard.T (stationary) @ xT (moving)
  - RoPE in [j, tok] layout: weight rows host-permuted per head to
    [evens 0:16 | odds 0:16 | evens 16:32 | odds 16:32] so the rotation
    partner lives 16 partitions away -> one DVE stream_shuffle; the
    PSUM->SBUF cast copy also on DVE (ACT is the global bottleneck)
  - logits^T [k, q] = KT-slice (stationary, K=64) @ QT-slice; two heads run
    concurrently in PE row-groups 0:64 / 64:128; causally dead columns
    trimmed, diagonal 128x128 block masked by a 0/1 multiply post-exp
  - P = exp(0.125 * logits^T) on ACT straight out of PSUM (the only ACT
    work in the kernel; one activation table load)
  - attn^T [d, q] (+ sumexp row) = [V | ones] (stationary) @ P
  - 1/Z via DVE reciprocal_approx_fast (replaces the ACT ln/exp chain),
    partition-broadcast on GpSimd, normalize on DVE
  - o_proj: out[tok, n] = attn^T chunk (stationary) @ Wo-shard.T (moving),
    fp32 partial evicted by DVE and DMAd to DRAM
"""

import sys

if "/opt/trn_rl_repo" not in sys.path:
    sys.path.insert(0, "/opt/trn_rl_repo")

import numpy as np
import ml_dtypes

import concourse.bass as bass  # noqa: F401
import concourse.tile as tile
from concourse import bacc, mybir

F32 = mybir.dt.float32
BF16 = mybir.dt.bfloat16
AF = mybir.ActivationFunctionType
NPBF16 = np.dtype(ml_dtypes.bfloat16)

B = 2
S = 2048
D_MODEL = 1024
N_HEADS = 16
D_K = 64
THETA = 10000.0

H_PER = 4          # heads per core
JW = H_PER * D_K   # 256: per-core projection width
N_CORES = 8
VSTRIDE = D_K + 1  # V tile col stride per head (64 data + 1 ones)
NDC = D_MODEL // 128  # 8 d-chunks

SWAP_MASK = list(range(16, 32)) + list(range(16))  # exchange 16-halves


def _act(nc, out, in_, func, scale=1.0):
    """ACT activation: out = func(in_*scale)."""
    return nc.scalar.activation(out, in_, func, bias=0.0, scale=float(scale))


_tables_pinned = False


def _pin_act_table():
    """Make every ACT func we emit resolve to the single table
    natural_log_exp_and_others so exactly one table load happens."""
    global _tables_pinned
    if _tables_pinned:
        return
    _tables_pinned = True
    import concourse.bacc as bacc_mod

    orig = bacc_mod.get_activation_tables
    keep = "natural_log_exp_and_others"
    ours = {AF.Exp, AF.Ln, AF.Copy, AF.Identity}

    def pinned(arch):
        t = orig(arch)
        return {
            name: (funcs if name == keep else funcs - ours)
            for name, funcs in t.items()
        }

    bacc_mod.get_activation_tables = pinned


def _build_program():
    _pin_act_table()
    nc = bacc.Bacc("TRN2", target_bir_lowering=False, debug=False)

    xT = nc.dram_tensor("xT", [D_MODEL, S], BF16, kind="ExternalInput")
    wq = nc.dram_tensor("wq", [D_MODEL, JW], BF16, kind="ExternalInput")
    wk = nc.dram_tensor("wk", [D_MODEL, JW], BF16, kind="ExternalInput")
    wv = nc.dram_tensor("wv", [D_MODEL, JW], BF16, kind="ExternalInput")
    wo = nc.dram_tensor("wo", [JW, D_MODEL], BF16, kind="ExternalInput")
    cost = nc.dram_tensor("cost", [128, S], BF16, kind="ExternalInput")
    sint = nc.dram_tensor("sint", [128, S], BF16, kind="ExternalInput")
    maskt = nc.dram_tensor("maskt", [128, 128], BF16, kind="ExternalInput")
    outp = nc.dram_tensor("outp", [S, D_MODEL], BF16, kind="ExternalOutput")

    with tile.TileContext(nc) as tc:
        _body(tc, xT, wq, wk, wv, wo, cost, sint, maskt, outp)
    nc.compile()
    return nc


def _body(tc, xT, wq, wk, wv, wo, cost, sint, maskt, outp):
    nc = tc.nc

    with (
        tc.tile_pool(name="const", bufs=1) as cpool,
        tc.tile_pool(name="big", bufs=1) as bpool,
        tc.tile_pool(name="ropep", bufs=3) as ropep,
        tc.tile_pool(name="pp", bufs=4) as pp,
        tc.tile_pool(name="np_", bufs=2) as npool,
        tc.tile_pool(name="op", bufs=3) as op,
        tc.tile_pool(name="ps", space="PSUM", bufs=1) as psp,
    ):
        # --- resident inputs: whole-tensor DMAs, spread across queues so
        # the first projection chain's operands (x chunk 0, wq) land first.
        xfull = bpool.tile([128, NDC, S], BF16, name="xfull")
        wq_sb = cpool.tile([128, NDC, JW], BF16, name="wq_sb")
        wk_sb = cpool.tile([128, NDC, JW], BF16, name="wk_sb")
        wv_sb = cpool.tile([128, NDC, JW], BF16, name="wv_sb")
        wo_sb = cpool.tile([128, 2, D_MODEL], BF16, name="wo_sb")
        cs_sb = cpool.tile([128, 2, S], BF16, name="cs_sb")
        mask_sb = cpool.tile([128, 128], BF16, name="mask_sb")

        # priority: x chunks + wq/wk + cos/sin(token-block 0) first (gates the
        # first Q/K chains + rope), then wv, remaining cos/sin, wo, mask.
        # NOTHING on the scalar queue: a dma_start issue occupies that
        # engine's sequencer, and ACT (exp) is the critical engine.
        # x is DMAd per (d-chunk, token-block) so the first Q/K chains (which
        # only contract block-0 columns) start ~4us in instead of waiting for
        # the whole 4MB of x.
        nc.gpsimd.dma_start(wq_sb[:], wq.rearrange("(c p) j -> p c j", p=128))
        nc.sync.dma_start(wk_sb[:], wk.rearrange("(c p) j -> p c j", p=128))
        qs = [nc.sync, nc.gpsimd]
        for dc in range(NDC):
            bsl = slice(0, 512)
            qs[dc % 2].dma_start(xfull[:, dc, bsl],
                                 xT[dc * 128:(dc + 1) * 128, bsl])
        nc.sync.dma_start(cs_sb[:, 0, 0:512], cost[:, 0:512])
        nc.gpsimd.dma_start(cs_sb[:, 1, 0:512], sint[:, 0:512])
        for dc in range(NDC):
            bsl = slice(512, 1024)
            qs[(dc + 1) % 2].dma_start(xfull[:, dc, bsl],
                                       xT[dc * 128:(dc + 1) * 128, bsl])
        nc.gpsimd.dma_start(wv_sb[:], wv.rearrange("(c p) j -> p c j", p=128))
        nc.sync.dma_start(mask_sb[:], maskt[:])
        for b in range(2, 4):
            for dc in range(NDC):
                bsl = slice(b * 512, (b + 1) * 512)
                qs[(dc + b) % 2].dma_start(xfull[:, dc, bsl],
                                           xT[dc * 128:(dc + 1) * 128, bsl])
        for tt in range(1, 4):
            tsl = slice(tt * 512, (tt + 1) * 512)
            nc.sync.dma_start(cs_sb[:, 0, tsl], cost[:, tsl])
            nc.gpsimd.dma_start(cs_sb[:, 1, tsl], sint[:, tsl])
        nc.gpsimd.dma_start(wo_sb[:], wo.rearrange("(c p) n -> p c n", p=128))

        # --- persistent activations ---
        qt_sb = bpool.tile([128, 2, S], BF16, name="qt_sb")   # [j, jg, tok]
        kt_sb = bpool.tile([128, 2, S], BF16, name="kt_sb")
        v_sb = bpool.tile([128, S // 128, H_PER * VSTRIDE], BF16, name="v_sb")
        at_sb = bpool.tile([128, 2, S], BF16, name="at_sb")   # attn^T normalized

        # ones columns for the fused softmax denominator
        for h in range(H_PER):
            nc.vector.memset(v_sb[:, :, h * VSTRIDE + D_K], 1.0)

        # HAM warmup scratch: dependency-free matmuls through a pw slot keep
        # the PE busy while the startup DMAs land.
        wsc = cpool.tile([128, 512], BF16, name="wsc")
        nc.vector.memset(wsc[:], 0.0)

        def pe_filler(n=1, free=512):
            wps = psp.tile([128, 512], F32, name=f"wps{pe_filler.i}",
                           tag="pw", bufs=2)
            pe_filler.i += 1
            for _ in range(n):
                nc.tensor.matmul(wps[:, 0:free], wsc[:, 0:128],
                                 wsc[:, 0:free], start=True, stop=True)
        pe_filler.i = 0

        # ---------------- pump machinery ----------------
        # proj/oproj work is emitted as small closures ("units") drained a
        # couple at a time inside the attention kt loop, so the PE queue
        # never holds a long projection run in front of the logits that
        # feed the ACT exp stream (the critical engine).
        pump_q = []   # entries: (label, closure)

        def pump(n, backstop=False):
            for _ in range(n):
                if not pump_q:
                    if backstop:
                        pe_filler(1)   # keep HAM at 8/8 in ACT-bound tail
                    return
                pump_q.pop(0)[1]()

        def queue(lbl, units):
            pump_q.extend((lbl, u) for u in units)

        def flush_label(lbl):
            while any(l == lbl for l, _ in pump_q):
                pump_q.pop(0)[1]()

        def flush_pump():
            while pump_q:
                pump_q.pop(0)[1]()

        def qk_pair(tt, jg):
            """Unit list: Q+K projection chains for (token block tt, head
            pair jg), then their RoPE."""
            tsl = slice(tt * 512, (tt + 1) * 512)
            st = {}

            def mk_mm(dcs):
                def u():
                    if "chains" not in st:
                        st["chains"] = []
                        for wsb, dst, pnm in ((wq_sb, qt_sb, "q"),
                                              (wk_sb, kt_sb, "k")):
                            ps = psp.tile([128, 512], F32,
                                          name=f"ps{pnm}{tt}{jg}",
                                          tag="pw", bufs=2)
                            st["chains"].append((ps, wsb, dst))
                    for dc in dcs:
                        for ps, wsb, dst in st["chains"]:
                            nc.tensor.matmul(
                                ps[:],
                                wsb[:, dc, jg * 128:(jg + 1) * 128],
                                xfull[:, dc, tsl],
                                start=(dc == 0), stop=(dc == NDC - 1),
                                skip_group_check=True,
                            )
                return u

            def rope_u():
                for ci, (ps, wsb, dst) in enumerate(st["chains"]):
                    # RoPE: dst = eq*cos + shuffle16(eq)*sin', value and its
                    # shuffle packed so one strided mul covers both products
                    eq = ropep.tile([128, 2, 512], BF16,
                                    name=f"eq{tt}{jg}{ci}", tag="eq")
                    nc.vector.tensor_copy(eq[:, 0, :], ps[:])
                    nc.vector.stream_shuffle(eq[:, 1, :], eq[:, 0, :],
                                             SWAP_MASK)
                    t12 = ropep.tile([128, 2, 512], BF16,
                                     name=f"t12{tt}{jg}{ci}", tag="t12")
                    nc.vector.tensor_mul(t12[:], eq[:], cs_sb[:, :, tsl])
                    nc.vector.tensor_add(dst[:, jg, tsl],
                                         t12[:, 0, :], t12[:, 1, :])

            # 2-dc units: fine enough to interleave with attention, coarse
            # enough that the PE reorder window still hides LDWEIGHTS
            return [mk_mm([0, 1]), mk_mm([2, 3]), mk_mm([4, 5]),
                    mk_mm([6, 7]), rope_u]

        def v_pair(tt, stp):
            """Unit list: V projection for two 128-token subtiles."""
            st = {}

            def mk_mm(dcs):
                def u():
                    if "vts" not in st:
                        st["vts"] = []
                        for sti in range(2):
                            s2 = 2 * stp + sti
                            psv = psp.tile([128, JW], F32,
                                           name=f"psv{tt}{s2}",
                                           tag="pw", bufs=2)
                            st["vts"].append((s2, psv))
                    for dc in dcs:
                        for s2, psv in st["vts"]:
                            nc.tensor.matmul(
                                psv[:],
                                xfull[:, dc, tt * 512 + s2 * 128:
                                      tt * 512 + (s2 + 1) * 128],
                                wv_sb[:, dc, :],
                                start=(dc == 0), stop=(dc == NDC - 1),
                                skip_group_check=True,
                            )
                return u

            def ev_u():
                for s2, psv in st["vts"]:
                    ktile = tt * 4 + s2
                    # one strided copy fills all 4 heads' 64-col runs
                    nc.vector.tensor_copy(
                        v_sb[:, ktile, :]
                        .rearrange("p (h s) -> p h s", h=H_PER)[:, :, 0:D_K],
                        psv[:].rearrange("p (h s) -> p h s", h=H_PER),
                    )

            return [mk_mm([0, 1]), mk_mm([2, 3]), mk_mm([4, 5]),
                    mk_mm([6, 7]), ev_u]

        def attention(hp, qt, pump_n=2, backstop=False):
            qsl = slice(qt * 512, (qt + 1) * 512)
            nkt = 4 * qt + 4
            nfull = 4 * qt
            pat = psp.tile([D_K + 1, 2, 512], F32,
                           name=f"pat{hp}{qt}", tag="pat", bufs=1)
            for kt in range(nkt):
                r = kt - nfull
                c0 = 128 * r if r >= 0 else 0
                psl = psp.tile([128, 2, 512], F32,
                               name=f"psl{hp}{qt}{kt}", tag="psl", bufs=2)
                p = pp.tile([128, 2, 512], BF16,
                            name=f"p{hp}{qt}{kt}", tag="p", bufs=8)
                for hh in range(2):
                    rows = slice(hh * 64, hh * 64 + 64)
                    nc.tensor.matmul(
                        psl[:, hh, c0:],
                        kt_sb[rows, hp, kt * 128:(kt + 1) * 128],
                        qt_sb[rows, hp, qt * 512 + c0:(qt + 1) * 512],
                        start=True, stop=True,
                    )
                _act(nc, p[:, :, c0:], psl[:, :, c0:], AF.Exp, scale=0.125)
                if r >= 0:
                    nc.vector.tensor_mul(
                        p[:, :, c0:c0 + 128],
                        p[:, :, c0:c0 + 128],
                        mask_sb[:, None, :].broadcast_to((128, 2, 128)),
                    )
                for hh in range(2):
                    h = 2 * hp + hh
                    nc.tensor.matmul(
                        pat[:, hh, c0:],
                        v_sb[:, kt, h * VSTRIDE:h * VSTRIDE + VSTRIDE],
                        p[:, hh, c0:],
                        start=(kt == 0), stop=(kt == nkt - 1),
                        skip_group_check=True,
                    )
                pump(pump_n, backstop)
            # normalize: at = pat[0:64] * bcast(exp(-ln pat[64])); ln and exp
            # share one activation table so no table reloads
            rf = npool.tile([D_K + 1, 2, 512], F32, name=f"rf{hp}{qt}", tag="rf")
            _act(nc, rf[64:65, :, :], pat[64:65, :, :], AF.Ln)
            rz = npool.tile([D_K + 1, 2, 512], F32, name=f"rz{hp}{qt}", tag="rz")
            _act(nc, rz[64:65, :, :], rf[64:65, :, :], AF.Exp, scale=-1.0)
            r0 = npool.tile([1, 2, 512], F32, name=f"r0{hp}{qt}", tag="r0")
            nc.sync.dma_start(r0[:], rz[64:65, :, :])
            rb = npool.tile([64, 2, 512], F32, name=f"rb{hp}{qt}", tag="rb")
            nc.gpsimd.partition_broadcast(rb[:], r0[:])
            nc.vector.tensor_mul(
                at_sb[0:64, hp, qsl], pat[0:64, 0, :], rb[:, 0, :]
            )
            tmp = npool.tile([64, 512], BF16, name=f"att{hp}{qt}", tag="att")
            nc.vector.tensor_mul(tmp[:], pat[0:64, 1, :], rb[:, 1, :])
            nc.sync.dma_start(at_sb[64:128, hp, qsl], tmp[:])
            pump(pump_n, backstop)

        def oproj_units(qt):
            units = []
            for tb in range(4 * qt, 4 * qt + 4):
                st = {}
                for nd in range(2):
                    def u(tb=tb, nd=nd, st=st):
                        rsl = slice(tb * 128, (tb + 1) * 128)
                        if nd == 0:
                            st["oev"] = op.tile([128, D_MODEL], BF16,
                                                name=f"oev{tb}", tag="oev")
                        oev = st["oev"]
                        pso = psp.tile([128, 512], F32, name=f"pso{tb}{nd}",
                                       tag="pw", bufs=2)
                        for hc in range(2):
                            nc.tensor.matmul(
                                pso[:],
                                at_sb[:, hc, rsl],
                                wo_sb[:, hc, nd * 512:(nd + 1) * 512],
                                start=(hc == 0), stop=(hc == 1),
                                skip_group_check=True,
                            )
                        nc.vector.tensor_copy(oev[:, nd * 512:(nd + 1) * 512],
                                              pso[:])
                        if nd == 1:
                            (nc.sync if tb % 2 == 0 else nc.gpsimd).dma_start(
                                outp[rsl, :], oev[:])
                    units.append(u)
            return units

        def oproj_tail(qt, hc):
            # two-pass tail: the hc=0 half depends only on the first
            # head-pair's normalize, so it runs under the second pair's
            # attention (as pump units); hc=1 lands after, merged on DVE.
            units = []
            for tb in range(4 * qt, 4 * qt + 4):
                def u(tb=tb):
                    rsl = slice(tb * 128, (tb + 1) * 128)
                    if hc == 0:
                        oev = op.tile([128, D_MODEL], F32, name=f"oevt{tb}",
                                      tag="oevt", bufs=4)
                        oproj_tail.oevs[tb] = oev
                    else:
                        oev = oproj_tail.oevs[tb]
                        oevb = op.tile([128, D_MODEL], BF16,
                                       name=f"oevb{tb}", tag="oevb", bufs=4)
                    for nd in range(2):
                        pso = psp.tile([128, 512], F32,
                                       name=f"psot{tb}{nd}{hc}",
                                       tag="pw", bufs=2)
                        nc.tensor.matmul(
                            pso[:],
                            at_sb[:, hc, rsl],
                            wo_sb[:, hc, nd * 512:(nd + 1) * 512],
                            start=True, stop=True, skip_group_check=True,
                        )
                        if hc == 0:
                            nc.vector.tensor_copy(
                                oev[:, nd * 512:(nd + 1) * 512], pso[:])
                        else:
                            nc.vector.tensor_add(
                                oevb[:, nd * 512:(nd + 1) * 512],
                                oev[:, nd * 512:(nd + 1) * 512], pso[:])
                    if hc == 1:
                        (nc.sync if tb % 2 == 0 else nc.gpsimd).dma_start(
                            outp[rsl, :], oevb[:])
                units.append(u)
            return units
        oproj_tail.oevs = {}

        # ---------------- fused schedule ----------------
        # proj(0)'s first half runs eagerly (gates the very first exp);
        # everything else drains through the pump inside attention loops at
        # the latest dependency-safe point: qk(qt,0) before attention(0,qt),
        # qk(qt,1) before attention(1,qt), V(qt) by the diagonal PV steps,
        # and all of o_proj inside qt=3 (the ACT-bound tail's PE slack).
        nq = S // 512
        # startup HAM warm-up: short N=256 fillers — enough sustained PE
        # activity to release the clock gate (~3.4us) without queueing so
        # much always-ready work that the first projection chains (gated on
        # the x DMA) get pushed behind it at cold clock.
        for _ in range(12):
            pe_filler(2, free=256)
        for u in qk_pair(0, 0):
            u()
        for u in v_pair(0, 0) + v_pair(0, 1):
            u()
        queue("qk01", qk_pair(0, 1))
        # boost qt=0 attention above the eagerly-emitted V/pump matmuls so
        # the very first logits->exp chain wins the PE the moment RoPE
        # lands (both are ready at the same scheduler tick; emission
        # priority would otherwise queue ~40 projection MMs first, at cold
        # clock).
        with tc.high_priority(offset=2000):
            attention(0, 0, pump_n=2)
            flush_label("qk01")
            queue("qk10", qk_pair(1, 0))
            attention(1, 0, pump_n=2)
        flush_label("qk10")
        queue("v1", v_pair(1, 0) + v_pair(1, 1))
        queue("qk11", qk_pair(1, 1))
        attention(0, 1, pump_n=2)
        flush_label("qk11")
        queue("qk20", qk_pair(2, 0))
        attention(1, 1, pump_n=2)
        flush_label("qk20")
        queue("v2", v_pair(2, 0) + v_pair(2, 1))
        queue("qk21", qk_pair(2, 1))
        attention(0, 2, pump_n=2)
        flush_label("qk21")
        queue("qk30", qk_pair(3, 0))
        attention(1, 2, pump_n=2)
        flush_label("qk30")
        queue("v3", v_pair(3, 0) + v_pair(3, 1))
        queue("qk31", qk_pair(3, 1))
        attention(0, nq - 1, pump_n=1, backstop=True)
        flush_label("qk31")
        queue("op0", oproj_units(0))
        queue("op1", oproj_units(1))
        queue("op2", oproj_units(2))
        queue("tail0", oproj_tail(nq - 1, 0))
        attention(1, nq - 1, pump_n=1, backstop=True)
        flush_pump()
        pe_filler(8)   # bridge the final normalize chain
        pe_filler(8)
        for u in oproj_tail(nq - 1, 1):
            u()


# ---------------------------------------------------------------------------
# host-side sharding / tables
# ---------------------------------------------------------------------------

def _head_perm_and_freq():
    """Within-head row order [e0..e15 | o0..o15 | e16..e31 | o16..o31]
    (e_i = dim 2i, o_i = dim 2i+1) so the rope partner is 16 partitions away
    inside one 32-partition quadrant. Returns (perm, freq_idx, sin_sign)."""
    e = np.arange(0, D_K, 2)   # evens: x1, freq i = 0..31
    o = np.arange(1, D_K, 2)   # odds:  x2
    perm = np.concatenate([e[:16], o[:16], e[16:], o[16:]])
    freq = np.concatenate([np.arange(16), np.arange(16),
                           np.arange(16, 32), np.arange(16, 32)])
    sign = np.concatenate([-np.ones(16), np.ones(16),
                           -np.ones(16), np.ones(16)])
    return perm, freq, sign


def _rope_tables():
    half = D_K // 2
    inv_freq = THETA ** (-np.arange(half, dtype=np.float64) * 2.0 / D_K)
    ang = np.arange(S, dtype=np.float64)[None, :] * inv_freq[:, None]  # [32, S]
    cos32 = np.cos(ang)
    sin32 = np.sin(ang)
    _, freq, sign = _head_perm_and_freq()
    cos64 = cos32[freq]                      # [64, S]
    sin64 = sin32[freq] * sign[:, None]      # [64, S]
    cos128 = np.tile(cos64, (2, 1)).astype(NPBF16)
    sin128 = np.tile(sin64, (2, 1)).astype(NPBF16)
    return cos128, sin128


def _mask_table():
    kl = np.arange(128)[:, None]
    ql = np.arange(128)[None, :]
    return np.ascontiguousarray((ql >= kl).astype(NPBF16))


_nc_cache = None


def _get_nc():
    global _nc_cache
    if _nc_cache is None:
        _nc_cache = _build_program()
    return _nc_cache


def make_in_maps(x, Wq, Wk, Wv, Wo):
    x = np.asarray(x, dtype=np.float32)
    Wq = np.asarray(Wq, dtype=np.float32)
    Wk = np.asarray(Wk, dtype=np.float32)
    Wv = np.asarray(Wv, dtype=np.float32)
    Wo = np.asarray(Wo, dtype=np.float32)

    cos128, sin128 = _rope_tables()
    mask = _mask_table()
    perm, _, _ = _head_perm_and_freq()

    in_maps = []
    for c in range(N_CORES):
        b = c // 4
        hg = c % 4
        heads = np.arange(hg * H_PER, (hg + 1) * H_PER)
        rows_plain = (heads[:, None] * D_K + np.arange(D_K)[None, :]).reshape(-1)
        rows_perm = (heads[:, None] * D_K + perm[None, :]).reshape(-1)
        in_maps.append({
            "xT": np.ascontiguousarray(x[b].T).astype(NPBF16),
            "wq": np.ascontiguousarray(Wq[rows_perm, :].T).astype(NPBF16),
            "wk": np.ascontiguousarray(Wk[rows_perm, :].T).astype(NPBF16),
            "wv": np.ascontiguousarray(Wv[rows_plain, :].T).astype(NPBF16),
            "wo": np.ascontiguousarray(Wo[:, rows_plain].T).astype(NPBF16),
            "cost": cos128,
            "sint": sin128,
            "maskt": mask,
        })
    return in_maps


def gather_output(results):
    outs = [np.asarray(r["outp"], dtype=np.float32) for r in results]
    out = np.stack([
        outs[0] + outs[1] + outs[2] + outs[3],
        outs[4] + outs[5] + outs[6] + outs[7],
    ])
    return out.reshape(B, S, D_MODEL)


def _install_ntff_hook():
    """Provide antenv.axon_hooks + register the ctypes NTFF profile hook.

    The agent image's antenv package lacks axon_hooks, so trace=True under
    axon crashes on import. Recreate the tiny get/set module and drive
    profiling via direct ctypes calls into libaxon_pjrt.so (same ABI as
    trn_boot._ntff_profile_via_ctypes)."""
    import types
    import ctypes
    import contextlib

    if "antenv.axon_hooks" not in sys.modules:
        mod = types.ModuleType("antenv.axon_hooks")
        mod._hook = None

        def set_axon_ntff_profile_hook(h):
            mod._hook = h

        def get_axon_ntff_profile_hook():
            return mod._hook

        mod.set_axon_ntff_profile_hook = set_axon_ntff_profile_hook
        mod.get_axon_ntff_profile_hook = get_axon_ntff_profile_hook
        sys.modules["antenv.axon_hooks"] = mod
        import antenv

        antenv.axon_hooks = mod

    hooks = sys.modules["antenv.axon_hooks"]
    if hooks.get_axon_ntff_profile_hook() is not None:
        return

    so_path = "/opt/axon/libaxon_pjrt.so"
    try:
        lib = ctypes.CDLL(so_path)
    except OSError:
        return
    if not hasattr(lib, "axon_start_nrt_profile"):
        return
    lib.axon_start_nrt_profile.argtypes = [
        ctypes.POINTER(ctypes.c_int64), ctypes.c_size_t,
    ]
    lib.axon_start_nrt_profile.restype = ctypes.c_int64
    lib.axon_stop_nrt_profile.argtypes = [ctypes.c_char_p]
    lib.axon_stop_nrt_profile.restype = ctypes.c_int64

    @contextlib.contextmanager
    def _hook(output_dir, device_ids):
        import jax

        jax.devices()
        if device_ids:
            ids = (ctypes.c_int64 * len(device_ids))(*device_ids)
            rc = lib.axon_start_nrt_profile(ids, len(device_ids))
        else:
            rc = lib.axon_start_nrt_profile(None, 0)
        if rc != 0:
            raise RuntimeError(f"axon_start_nrt_profile rc={rc}")
        try:
            yield
        finally:
            n = lib.axon_stop_nrt_profile(str(output_dir).encode())
            print(f"profile: {n} file(s) written to {output_dir}")

    hooks.set_axon_ntff_profile_hook(_hook)


def kernel(x, Wq, Wk, Wv, Wo, _trace=False, _trace_cores=None):
    from concourse.bass_utils import run_bass_kernel_spmd

    if _trace:
        _install_ntff_hook()
    nc = _get_nc()
    in_maps = make_in_maps(x, Wq, Wk, Wv, Wo)
    res = run_bass_kernel_spmd(
        nc, in_maps, list(range(N_CORES)),
        trace=_trace, trace_cores=_trace_cores,
    )
    out = gather_output(res.results)
    if _trace:
        kernel.last_results = res
    return out
